# revision 9
# baseline (speedup 1.0000x reference)
"""Trainium2 Bass kernel for nn_BaseModel_14499809591724 (GNN message passing).

Strategy (8 NeuronCores, data-parallel over graph batches):
  - Nodes split into 8 contiguous shards at graph boundaries (batch sorted),
    padded to S=6400 rows each; replicated node table = [8*S, 128] bf16 in
    DRAM, chunk-major in 2 chunks of 25600 rows (int16 gather-idx limit).
  - Self-loops folded in as virtual edges (v,v) with weight 1/deg[v], so
    aggregation is one edge sweep with no per-window fixup.
  - Each core owns the edges whose dst is in its shard. Edges sorted by
    (src-chunk k, 5-window block, dst window, slot) and padded into groups of
    128. Per conv: one dma_gather per (k, block) fetches all needed src rows
    (round-robin over 4 SWDGE queues); DVE builds a scaled one-hot
    [e=partition, dst] per group; PE matmul (stationary=G group, moving=oh)
    accumulates feature-major agg^T[f, dst] in PSUM per (window, k).
  - The conv weight matmul absorbs both chunk partials: hn = W^T agg0 +
    W^T agg1 accumulated in PSUM; Relu+bias on Act -> feature-major h. Convs
    that feed another conv transpose back to node-major and DMA into the
    AllGather staging buffer; the table AG runs in 2 chunks so conv l+1's
    chunk-0 work overlaps AG chunk 1.
  - JumpingKnowledge + per-graph pooling (one-hot matmul) + BN + MLP head +
    softmax run per core on its own 64 graphs; host concatenates 8 x [64, 10].
"""
import sys
import numpy as np
import ml_dtypes

sys.path.insert(0, "/opt/trn_rl_repo")

from concourse import bacc, tile, mybir  # noqa: E402
from concourse.bass_utils import run_bass_kernel_spmd  # noqa: E402

# ---- model / sharding constants (shapes fixed by the problem) ----
NC = 8
N_NODES = 50000
N_EDGES = 800000
F = 128
B = 512
GPC = B // NC          # graphs per core = 64
S = 6400               # padded nodes per shard (max real shard is 6368)
NW = S // 128          # 50 windows per core
TAB = NC * S           # 51200 table rows
NCH = 2                # table chunks (AllGather pipeline + int16 idx limit)
CHS = S // NCH         # 3200 shard rows per chunk
CHROWS = NC * CHS      # 25600 table rows per chunk (int16-safe)
WPB = 5                # windows per gather block
NBLK = NW // WPB       # 10 blocks
NB = 3
BN_EPS = 1e-5
NSWQ = 4               # SWDGE queues for gathers

f32 = mybir.dt.float32
bf16 = mybir.dt.bfloat16
i16 = mybir.dt.int16

_PROGRAM = {}


def _ch_of(w, k, ch_own, ch_oth):
    """Groups per (window, src-chunk): self edges land in the window's own
    chunk (windows 0-24 -> chunk 0), so that bucket gets the larger CH."""
    return ch_own if (w < NW // 2) == (k == 0) else ch_oth


def _layout(ch_own, ch_oth):
    """Emission-order layout: group column and idx column offsets.

    Returns (gcol_of[(k,w)], icol_of[(k,blk)], g_tot, ic_tot); groups are
    ordered (k, blk, w, c); idxs are 128 per group, wrapped 16-partition."""
    gcol_of, icol_of = {}, {}
    gcol = icol = 0
    for k in range(NCH):
        for blk in range(NBLK):
            icol_of[(k, blk)] = icol
            for wi in range(WPB):
                w = blk * WPB + wi
                ch = _ch_of(w, k, ch_own, ch_oth)
                gcol_of[(k, w)] = gcol
                gcol += ch
                icol += ch * 8  # ch*128 idxs / 16 partitions
    return gcol_of, icol_of, gcol, icol


def _wrap_run(run):
    """[n] idx run -> [16, n//16] int16 (idx p -> partition p%16, col p//16)."""
    n = run.shape[0]
    return run.reshape(n // 16, 16).T.astype(np.int16)


def _preprocess(inp: dict):
    batch = np.asarray(inp["batch"])
    ei = np.asarray(inp["edge_index"])
    ew = np.asarray(inp["edge_attr"], dtype=np.float32)
    x = np.asarray(inp["x"], dtype=np.float32)
    src, dst = ei[0].astype(np.int64), ei[1].astype(np.int64)

    bounds = np.searchsorted(batch, np.arange(0, B + 1, GPC)).astype(np.int64)
    sizes = np.diff(bounds)
    assert sizes.max() <= S, f"shard overflow: {sizes.max()} > {S}"

    node = np.arange(N_NODES, dtype=np.int64)
    core_of = (np.searchsorted(bounds, node, side="right") - 1).astype(np.int64)
    off = node - bounds[core_of]
    # chunk-major table: row = chunk*CHROWS + core*CHS + (off % CHS)
    tab = (off // CHS) * CHROWS + core_of * CHS + (off % CHS)

    deg = (np.bincount(dst, weights=ew.astype(np.float64), minlength=N_NODES) + 1.0)
    deg = deg.astype(np.float32)
    dinv = 1.0 / np.sqrt(deg)
    norm = (dinv[src] * ew * dinv[dst]).astype(np.float32)
    dinv2 = (1.0 / deg).astype(np.float32)

    # self-loops as virtual edges (v, v, 1/deg[v])
    src = np.concatenate([src, node])
    dst = np.concatenate([dst, node])
    norm = np.concatenate([norm, dinv2])

    # full replicated x table (node-major, bf16)
    xtab = np.zeros((TAB, F), dtype=ml_dtypes.bfloat16)
    xtab[tab] = x.astype(ml_dtypes.bfloat16)

    iota = np.tile(np.arange(128, dtype=np.float32), (128, 1)).astype(ml_dtypes.bfloat16)
    identf = np.eye(128, dtype=np.float32)
    identb = np.eye(128, dtype=ml_dtypes.bfloat16)

    # weights
    conv_w = np.asarray(inp["conv_w"], dtype=np.float32).reshape(6, F, F)
    convw = conv_w.transpose(1, 0, 2).reshape(F, 6 * F).astype(ml_dtypes.bfloat16)
    convb = np.asarray(inp["conv_b"], dtype=np.float32).reshape(6, F).T.copy()
    jk_w = np.asarray(inp["jk_w"], dtype=np.float32).reshape(NB, 2, F, F).reshape(6, F, F)
    jkw = jk_w.transpose(1, 0, 2).reshape(F, 6 * F).astype(ml_dtypes.bfloat16)
    jkb = np.asarray(inp["jk_b"], dtype=np.float32).T.copy()
    s = (np.asarray(inp["bn_gamma"], dtype=np.float32)
         / np.sqrt(np.asarray(inp["bn_var"], dtype=np.float32) + BN_EPS))
    t = (np.asarray(inp["bn_beta"], dtype=np.float32)
         - np.asarray(inp["bn_mean"], dtype=np.float32) * s)
    bns = s.reshape(NB, F).T.copy()
    bnt = t.reshape(NB, F).T.copy()
    lin1_w = np.asarray(inp["lin1_w"], dtype=np.float32).reshape(NB, F, F)
    l1w = lin1_w.transpose(1, 0, 2).reshape(F, NB * F).copy()
    l1b = np.asarray(inp["lin1_b"], dtype=np.float32).reshape(F, 1).copy()
    l2w = np.asarray(inp["lin2_w"], dtype=np.float32).copy()
    l2b = np.asarray(inp["lin2_b"], dtype=np.float32).reshape(10, 1).copy()

    shared = {
        "xtab": xtab, "iota": iota, "identf": identf, "identb": identb,
        "convw": convw, "convb": convb, "jkw": jkw, "jkb": jkb,
        "bns": bns, "bnt": bnt, "l1w": l1w, "l1b": l1b, "l2w": l2w, "l2b": l2b,
    }

    dst_core = core_of[dst]
    dst_off = off[dst]
    src_row = tab[src] % CHROWS
    src_chunk = tab[src] // CHROWS

    # CH sizes from the worst (window, chunk) bucket over all cores
    e_w_all = dst_off // 128
    own = (e_w_all < NW // 2) == (src_chunk == 0)
    key_all = (dst_core * NW + e_w_all) * NCH + src_chunk
    cnt = np.bincount(key_all, minlength=NC * NW * NCH)
    cnt_own = np.bincount(key_all[own], minlength=NC * NW * NCH)
    ch_own = int(-(-int(cnt_own.max()) // 128))
    ch_oth = int(-(-int((cnt - cnt_own).max()) // 128))

    gcol_of, icol_of, g_tot, ic_tot = _layout(ch_own, ch_oth)

    in_maps = []
    for c in range(NC):
        eidx = np.flatnonzero(dst_core == c)
        e_w = dst_off[eidx] // 128
        e_k = src_chunk[eidx]
        key = (e_k * NBLK + e_w // WPB) * NW + e_w  # (k, blk, w)
        order = np.argsort(key, kind="stable")
        eidx = eidx[order]
        kw_cnt = np.bincount(e_k[order] * NW + e_w[order], minlength=NCH * NW)

        idx_cols = np.zeros((16, ic_tot), dtype=np.int16)
        rel_cols = np.zeros((128, g_tot), dtype=np.float32)
        nrm_cols = np.zeros((128, g_tot), dtype=np.float32)

        pos = 0
        for k in range(NCH):
            for blk in range(NBLK):
                icol = icol_of[(k, blk)]
                for wi in range(WPB):
                    w = blk * WPB + wi
                    ch = _ch_of(w, k, ch_own, ch_oth)
                    cap = ch * 128
                    n_e = int(kw_cnt[k * NW + w])
                    ee = eidx[pos: pos + n_e]
                    pos += n_e
                    idx_run = np.zeros(cap, dtype=np.int64)
                    idx_run[:n_e] = src_row[ee]
                    rel_run = np.zeros(cap, dtype=np.float32)
                    rel_run[:n_e] = (dst_off[ee] % 128).astype(np.float32)
                    nrm_run = np.zeros(cap, dtype=np.float32)
                    nrm_run[:n_e] = norm[ee]
                    idx_cols[:, icol: icol + cap // 16] = _wrap_run(idx_run)
                    icol += cap // 16
                    g0 = gcol_of[(k, w)]
                    rel_cols[:, g0: g0 + ch] = rel_run.reshape(ch, 128).T
                    nrm_cols[:, g0: g0 + ch] = nrm_run.reshape(ch, 128).T
        assert pos == len(eidx)
        gidx = np.tile(idx_cols, (8, 1))

        # per-node pooling one-hot
        ln = np.arange(sizes[c], dtype=np.int64)
        pool = np.zeros((128, NW * GPC), dtype=ml_dtypes.bfloat16)
        g_of = batch[bounds[c] + ln].astype(np.int64) - c * GPC
        pool[ln % 128, (ln // 128) * GPC + g_of] = 1.0

        m = {"gidx": gidx, "rel": rel_cols, "norm": nrm_cols, "pool": pool}
        m.update(shared)
        in_maps.append(m)
    return in_maps, (ch_own, ch_oth)


def _build_program(ch_own, ch_oth):
    gcol_of, icol_of, g_tot, ic_tot = _layout(ch_own, ch_oth)
    nc = bacc.Bacc("TRN2", target_bir_lowering=False, debug=False,
                   num_devices=NC, num_swdge_queues=NSWQ)
    AF = mybir.ActivationFunctionType
    OP = mybir.AluOpType

    ap = {}
    for name, shape, dt in [
        ("xtab", [TAB, F], bf16),
        ("gidx", [128, ic_tot], i16),
        ("rel", [128, g_tot], f32), ("norm", [128, g_tot], f32),
        ("pool", [128, NW * GPC], bf16),
        ("iota", [128, 128], bf16), ("identf", [128, 128], f32),
        ("identb", [128, 128], bf16),
        ("convw", [F, 6 * F], bf16), ("convb", [F, 6], f32),
        ("jkw", [F, 6 * F], bf16), ("jkb", [F, NB], f32),
        ("bns", [F, NB], f32), ("bnt", [F, NB], f32),
        ("l1w", [F, NB * F], f32), ("l1b", [F, 1], f32),
        ("l2w", [F, 10], f32), ("l2b", [10, 1], f32),
    ]:
        ap[name] = nc.dram_tensor(name, shape, dt, kind="ExternalInput").ap()
    out_ap = nc.dram_tensor("out", [GPC, 10], f32, kind="ExternalOutput").ap()

    with tile.TileContext(nc) as tc:
        with (
            tc.tile_pool(name="dram", bufs=1, space="DRAM") as dram,
            tc.tile_pool(name="pers", bufs=1) as pers,
            tc.tile_pool(name="rot", bufs=1) as rot,
            tc.tile_pool(name="psum", bufs=1, space="PSUM") as psum,
        ):
            ag_in = dram.tile([S, F], bf16)

            # ---- persistent SBUF loads
            sb = {}
            for name in ["gidx", "rel", "norm", "pool", "iota",
                         "identf", "identb", "convw", "convb", "jkw", "jkb",
                         "bns", "bnt", "l1w", "l1b", "l2w", "l2b"]:
                t_ = pers.tile(list(ap[name].shape), ap[name].dtype, name=f"sb_{name}")
                nc.sync.dma_start(t_[:], ap[name][:])
                sb[name] = t_

            h_nm = pers.tile([128, NW, F], bf16, name="h_nm")
            h1_fm = pers.tile([128, S], bf16, name="h1_fm")
            h2_fm = pers.tile([128, S], bf16, name="h2_fm")
            hb_fm = pers.tile([128, S], bf16, name="hb_fm")
            agg0 = pers.tile([128, NW, 128], bf16, name="agg0")
            z_sb = pers.tile([128, NB, GPC], f32, name="z_sb")

            qctr = [0]

            def conv(lk, tables, h_out, write_nm):
                for k in range(NCH):
                    for blk in range(NBLK):
                        ch = _ch_of(blk * WPB, k, ch_own, ch_oth)
                        nidx = WPB * ch * 128
                        ic0 = icol_of[(k, blk)]
                        G = rot.tile([128, WPB * ch_own, F], bf16, tag="G",
                                     bufs=3, name="G")
                        nc.gpsimd.dma_gather(
                            out_ap=G[:, :WPB * ch, :], in_ap=tables[k][:],
                            idxs_ap=sb["gidx"][:, ic0: ic0 + nidx // 16],
                            num_idxs=nidx, num_idxs_reg=nidx, elem_size=F,
                            single_packet=False, queue_num=qctr[0] % NSWQ)
                        qctr[0] += 1
                        for wi in range(WPB):
                            w = blk * WPB + wi
                            g0 = gcol_of[(k, w)]
                            pp = psum.tile([128, 128], f32, tag="agg",
                                           bufs=4, name="pp")
                            for c in range(ch):
                                col = g0 + c
                                oh = rot.tile([128, 128], bf16, tag="oh",
                                              bufs=6, name="oh")
                                nc.vector.tensor_scalar(
                                    out=oh[:], in0=sb["iota"][:],
                                    scalar1=sb["rel"][:, col:col + 1],
                                    scalar2=sb["norm"][:, col:col + 1],
                                    op0=OP.is_equal, op1=OP.mult)
                                nc.tensor.matmul(pp[:], G[:, wi * ch + c, :],
                                                 oh[:], start=(c == 0),
                                                 stop=(c == ch - 1))
                            if k == 0:
                                nc.scalar.copy(agg0[:, w, :], pp[:])
                            else:
                                a1 = rot.tile([128, 128], bf16, tag="a1",
                                              bufs=3, name="a1")
                                nc.scalar.copy(a1[:], pp[:])
                                hn = psum.tile([128, 128], f32, tag="hn",
                                               bufs=2, name="hn")
                                wsl = sb["convw"][:, lk * F:(lk + 1) * F]
                                nc.tensor.matmul(hn[:], wsl, agg0[:, w, :],
                                                 start=True, stop=False)
                                nc.tensor.matmul(hn[:], wsl, a1[:],
                                                 start=False, stop=True)
                                nc.scalar.activation(
                                    h_out[:, w * 128:(w + 1) * 128], hn[:],
                                    AF.Relu, bias=sb["convb"][:, lk:lk + 1])
                                if write_nm:
                                    hnT = psum.tile([128, 128], bf16, tag="hnT",
                                                    bufs=1, name="hnT")
                                    nc.tensor.transpose(
                                        hnT[:], h_out[:, w * 128:(w + 1) * 128],
                                        sb["identb"][:])
                                    nc.scalar.copy(h_nm[:, w, :], hnT[:])
                                    nc.sync.dma_start(
                                        ag_in[w * 128:(w + 1) * 128, :],
                                        h_nm[:, w, :])

            def allgather(i):
                tabs = []
                for k in range(NCH):
                    tk = dram.tile([CHROWS, F], bf16, addr_space="Shared",
                                   tag=f"t{i}_{k}", name=f"t{i}_{k}")
                    nc.gpsimd.collective_compute(
                        "AllGather", OP.bypass,
                        replica_groups=[list(range(NC))],
                        ins=[ag_in[k * CHS:(k + 1) * CHS, :].opt()],
                        outs=[tk.opt()])
                    tabs.append(tk)
                return tabs

            def jk(li, last):
                pooled = psum.tile([128, GPC], f32, tag="pooled", bufs=1,
                                   name="pooled")
                for w in range(NW):
                    hb = psum.tile([128, F], f32, tag="hn", bufs=2, name="hb")
                    nc.tensor.matmul(hb[:], sb["jkw"][:, (2 * li) * F:(2 * li + 1) * F],
                                     h1_fm[:, w * 128:(w + 1) * 128],
                                     start=True, stop=False)
                    nc.tensor.matmul(hb[:], sb["jkw"][:, (2 * li + 1) * F:(2 * li + 2) * F],
                                     h2_fm[:, w * 128:(w + 1) * 128],
                                     start=False, stop=True)
                    nc.scalar.activation(hb_fm[:, w * 128:(w + 1) * 128], hb[:],
                                         AF.Relu, bias=sb["jkb"][:, li:li + 1])
                    hnT = psum.tile([128, F], bf16, tag="hnT", bufs=1, name="hnT")
                    nc.tensor.transpose(hnT[:], hb_fm[:, w * 128:(w + 1) * 128],
                                        sb["identb"][:])
                    nc.scalar.copy(h_nm[:, w, :], hnT[:])
                    if not last:
                        nc.sync.dma_start(ag_in[w * 128:(w + 1) * 128, :],
                                          h_nm[:, w, :])
                    nc.tensor.matmul(pooled[:], h_nm[:, w, :],
                                     sb["pool"][:, w * GPC:(w + 1) * GPC],
                                     start=(w == 0), stop=(w == NW - 1))
                nc.scalar.copy(z_sb[:, li, :], pooled[:])

            # ---- main flow
            xtabs = [ap["xtab"][k * CHROWS:(k + 1) * CHROWS, :]
                     for k in range(NCH)]
            conv(0, xtabs, h1_fm, True)
            t = allgather(0)
            conv(1, t, h2_fm, False)
            jk(0, False)
            t = allgather(1)
            conv(2, t, h1_fm, True)
            t = allgather(2)
            conv(3, t, h2_fm, False)
            jk(1, False)
            t = allgather(3)
            conv(4, t, h1_fm, True)
            t = allgather(4)
            conv(5, t, h2_fm, False)
            jk(2, True)

            # ---- BN + MLP head + softmax
            zbn = rot.tile([128, NB, GPC], f32, tag="zbn", bufs=1, name="zbn")
            for ti in range(NB):
                nc.vector.tensor_scalar(
                    out=zbn[:, ti, :], in0=z_sb[:, ti, :],
                    scalar1=sb["bns"][:, ti:ti + 1], scalar2=sb["bnt"][:, ti:ti + 1],
                    op0=OP.mult, op1=OP.add)
            a1 = psum.tile([128, GPC], f32, tag="hn", bufs=2, name="a1h")
            for ti in range(NB):
                nc.tensor.matmul(a1[:], sb["l1w"][:, ti * F:(ti + 1) * F],
                                 zbn[:, ti, :], start=(ti == 0), stop=(ti == NB - 1))
            a1s = rot.tile([128, GPC], f32, tag="a1s", bufs=1, name="a1s")
            nc.scalar.activation(a1s[:], a1[:], AF.Relu, bias=sb["l1b"][:])
            z2 = psum.tile([10, GPC], f32, tag="pooled", bufs=1, name="z2")
            nc.tensor.matmul(z2[:], sb["l2w"][:], a1s[:], start=True, stop=True)
            z2s = rot.tile([10, GPC], f32, tag="z2s", bufs=1, name="z2s")
            nc.scalar.activation(z2s[:], z2[:], AF.Identity, bias=sb["l2b"][:])
            z2T = psum.tile([GPC, 10], f32, tag="hnT", bufs=1, name="z2T")
            nc.tensor.transpose(z2T[:], z2s[:], sb["identf"][0:10, 0:10])
            z2Ts = rot.tile([GPC, 10], f32, tag="z2Ts", bufs=1, name="z2Ts")
            nc.vector.tensor_copy(z2Ts[:], z2T[:])
            negm = rot.tile([GPC, 1], f32, tag="negm", bufs=1, name="negm")
            nc.vector.tensor_reduce(negm[:], z2Ts[:], mybir.AxisListType.X,
                                    OP.max, negate=True)
            et = rot.tile([GPC, 10], f32, tag="et", bufs=1, name="et")
            nc.scalar.activation(et[:], z2Ts[:], AF.Exp, bias=negm[:])
            ssum = rot.tile([GPC, 1], f32, tag="ssum", bufs=1, name="ssum")
            nc.vector.tensor_reduce(ssum[:], et[:], mybir.AxisListType.X, OP.add)
            rcp = rot.tile([GPC, 1], f32, tag="rcp", bufs=1, name="rcp")
            nc.vector.reciprocal(rcp[:], ssum[:])
            outt = rot.tile([GPC, 10], f32, tag="outt", bufs=1, name="outt")
            nc.vector.tensor_scalar_mul(outt[:], et[:], rcp[:])
            nc.sync.dma_start(out_ap[:], outt[:])

    nc.compile()
    return nc


def _get_program(layout):
    if layout not in _PROGRAM:
        _PROGRAM[layout] = _build_program(*layout)
    return _PROGRAM[layout]


def kernel(**inputs) -> np.ndarray:
    in_maps, layout = _preprocess(inputs)
    nc = _get_program(layout)
    res = run_bass_kernel_spmd(nc, in_maps, list(range(NC)))
    return np.concatenate([res.results[c]["out"] for c in range(NC)], axis=0)
